# revision 7
# baseline (speedup 1.0000x reference)
"""Trainium2 Bass kernel for nn_Aligner: conv projections -> -L2 distance GEMM
-> log_softmax + log prior -> softmax on device (8 NeuronCores, batch-parallel);
monotonic alignment search path on host from the device-computed attn_soft.

Self-contained: hardcodes shapes B=32, TD=1024, TE=384, D_ENC=512, D_DEC=80,
D_HID=256, ATTN_C=80.
"""
import os
import sys

import numpy as np

sys.path.insert(0, "/opt/trn_rl_repo")

from contextlib import ExitStack  # noqa: E402

import concourse.bass as bass  # noqa: E402
from concourse import bacc  # noqa: E402
import concourse.tile as tile  # noqa: E402
from concourse import mybir  # noqa: E402
from concourse.bass_utils import run_bass_kernel_spmd  # noqa: E402

LRELU = 0.3
TEMP = 0.0005
B, TD, TE = 32, 1024, 384
NCORES = 8
BS = B // NCORES  # samples per core

F32 = mybir.dt.float32
AF = mybir.ActivationFunctionType

_GRAPH = None
LAST_EXEC_NS = None


def _build_graph():
    nc = bacc.Bacc()

    enc = nc.dram_tensor("enc", (BS, 4, 128, TE), F32, kind="ExternalInput")
    dec = nc.dram_tensor("dec", (BS, 80, TD), F32, kind="ExternalInput")
    priorT = nc.dram_tensor("priorT", (BS, TD, TE), F32, kind="ExternalInput")
    w1 = nc.dram_tensor("w1", (128, 3 * 4 * 2 * 128), F32, kind="ExternalInput")
    b1 = nc.dram_tensor("b1", (128, 2), F32, kind="ExternalInput")
    w2 = nc.dram_tensor("w2", (128, 2 * 80), F32, kind="ExternalInput")
    b2 = nc.dram_tensor("b2", (80, 1), F32, kind="ExternalInput")
    u1 = nc.dram_tensor("u1", (80, 7 * 2 * 128), F32, kind="ExternalInput")
    c1 = nc.dram_tensor("c1", (128, 2), F32, kind="ExternalInput")
    u2 = nc.dram_tensor("u2", (128, 7 * 2 * 2 * 128), F32, kind="ExternalInput")
    c2 = nc.dram_tensor("c2", (128, 2), F32, kind="ExternalInput")
    u3 = nc.dram_tensor("u3", (128, 2 * 80), F32, kind="ExternalInput")
    c3 = nc.dram_tensor("c3", (80, 1), F32, kind="ExternalInput")
    o80 = nc.dram_tensor("o80", (80, 1), F32, kind="ExternalInput")
    o1r = nc.dram_tensor("o1r", (1, TE), F32, kind="ExternalInput")
    alp_out = nc.dram_tensor("alp", (BS, TD, TE), F32, kind="ExternalOutput")
    soft_out = nc.dram_tensor("soft", (BS, TD, TE), F32, kind="ExternalOutput")

    with tile.TileContext(nc) as tc, ExitStack() as ctx:
        wp = ctx.enter_context(tc.tile_pool(name="weights", bufs=1))
        sp = ctx.enter_context(tc.tile_pool(name="acts", bufs=2))
        pp = ctx.enter_context(tc.tile_pool(name="psum", bufs=2, space="PSUM"))

        w1t = wp.tile([128, 3 * 4 * 2 * 128], F32)
        nc.sync.dma_start(w1t[:], w1[:])
        w2t = wp.tile([128, 160], F32)
        nc.sync.dma_start(w2t[:], w2[:])
        u1t = wp.tile([80, 7 * 2 * 128], F32)
        nc.sync.dma_start(u1t[:], u1[:])
        u2t = wp.tile([128, 7 * 2 * 2 * 128], F32)
        nc.sync.dma_start(u2t[:], u2[:])
        u3t = wp.tile([128, 160], F32)
        nc.sync.dma_start(u3t[:], u3[:])
        b1t = wp.tile([128, 2], F32)
        nc.sync.dma_start(b1t[:], b1[:])
        b2t = wp.tile([80, 1], F32)
        nc.sync.dma_start(b2t[:], b2[:])
        c1t = wp.tile([128, 2], F32)
        nc.sync.dma_start(c1t[:], c1[:])
        c2t = wp.tile([128, 2], F32)
        nc.sync.dma_start(c2t[:], c2[:])
        c3t = wp.tile([80, 1], F32)
        nc.sync.dma_start(c3t[:], c3[:])
        o80t = wp.tile([80, 1], F32)
        nc.sync.dma_start(o80t[:], o80[:])
        o1t = wp.tile([1, TE], F32)
        nc.sync.dma_start(o1t[:], o1r[:])

        for s in range(BS):
            # ---- keys path: conv(k=3, 512->256) + lrelu, conv(1x1, 256->80) ----
            xe = sp.tile([128, 4, TE], F32, tag="xe")
            for c in range(4):
                nc.sync.dma_start(xe[:, c, :], enc[s, c])
            k1 = sp.tile([128, 2, TE], F32, tag="k1")
            for coc in range(2):
                pk = pp.tile([128, 512], F32, tag="psA")
                first = True
                for cic in range(4):
                    for dk in (1, 0, 2):
                        wi = ((dk * 4 + cic) * 2 + coc) * 128
                        if dk == 1:
                            o0, o1, i0 = 0, TE, 0
                        elif dk == 0:
                            o0, o1, i0 = 1, TE, 0
                        else:
                            o0, o1, i0 = 0, TE - 1, 1
                        nc.tensor.matmul(
                            pk[:, o0:o1],
                            w1t[:, wi : wi + 128],
                            xe[:, cic, i0 : i0 + (o1 - o0)],
                            start=first,
                            stop=(cic == 3 and dk == 2),
                        )
                        first = False
                tk = sp.tile([128, TE], F32, tag="tk")
                nc.vector.tensor_scalar(
                    tk[:], pk[:, :TE], b1t[:, coc : coc + 1], None,
                    mybir.AluOpType.add,
                )
                nc.vector.scalar_tensor_tensor(
                    k1[:, coc, :], tk[:], LRELU, tk[:],
                    mybir.AluOpType.mult, mybir.AluOpType.max,
                )
            ke = sp.tile([80, TE], F32, tag="ke")
            pk2 = pp.tile([80, 512], F32, tag="psB")
            for cic in range(2):
                nc.tensor.matmul(
                    pk2[:, :TE], w2t[:, cic * 80 : cic * 80 + 80], k1[:, cic, :],
                    start=(cic == 0), stop=(cic == 1),
                )
            nc.scalar.activation(ke[:], pk2[:, :TE], AF.Identity, bias=b2t[:, 0:1])
            ksq = sp.tile([80, TE], F32, tag="ksq")
            nc.scalar.activation(ksq[:], ke[:], AF.Square)
            pkr = pp.tile([1, 512], F32, tag="psS")
            nc.tensor.matmul(pkr[:, :TE], o80t[:], ksq[:], start=True, stop=True)
            k2row = sp.tile([1, TE], F32, tag="k2row")
            nc.scalar.activation(k2row[:], pkr[:, :TE], AF.Copy, scale=-0.5)

            # ---- queries path: conv(k=7, 80->256), conv(k=7, 256->256), 1x1 -> 80 ----
            xd = sp.tile([80, TD], F32, tag="xd")
            nc.sync.dma_start(xd[:], dec[s])
            q1 = sp.tile([128, 2, TD], F32, tag="q1")
            for coc in range(2):
                for tn in range(2):
                    pq = pp.tile([128, 512], F32, tag="psA")
                    first = True
                    for dk in (3, 0, 1, 2, 4, 5, 6):
                        g0 = max(0, 3 - dk)
                        g1 = TD + min(0, 3 - dk)
                        lo = max(g0, tn * 512)
                        hi = min(g1, (tn + 1) * 512)
                        if lo >= hi:
                            continue
                        wi = (dk * 2 + coc) * 128
                        nc.tensor.matmul(
                            pq[:, lo - tn * 512 : hi - tn * 512],
                            u1t[:, wi : wi + 128],
                            xd[:, lo + dk - 3 : hi + dk - 3],
                            start=first,
                            stop=(dk == 6),
                        )
                        first = False
                    tq = sp.tile([128, 512], F32, tag="tq")
                    nc.vector.tensor_scalar(
                        tq[:], pq[:], c1t[:, coc : coc + 1], None,
                        mybir.AluOpType.add,
                    )
                    nc.vector.scalar_tensor_tensor(
                        q1[:, coc, tn * 512 : (tn + 1) * 512], tq[:], LRELU, tq[:],
                        mybir.AluOpType.mult, mybir.AluOpType.max,
                    )
            q2 = sp.tile([128, 2, TD], F32, tag="q2")
            for coc in range(2):
                for tn in range(2):
                    pq = pp.tile([128, 512], F32, tag="psA")
                    first = True
                    segs = []
                    for cic in range(2):
                        for dk in (3, 0, 1, 2, 4, 5, 6):
                            g0 = max(0, 3 - dk)
                            g1 = TD + min(0, 3 - dk)
                            lo = max(g0, tn * 512)
                            hi = min(g1, (tn + 1) * 512)
                            if lo >= hi:
                                continue
                            # split so each matmul's rhs stays inside one
                            # 512-wide producer region of q1
                            cut = 512 + 3 - dk
                            pts = [lo, hi]
                            if lo < cut < hi:
                                pts = [lo, cut, hi]
                            for a, b in zip(pts[:-1], pts[1:]):
                                segs.append((cic, dk, a, b))
                    for idx, (cic, dk, a, b) in enumerate(segs):
                        wi = ((dk * 2 + cic) * 2 + coc) * 128
                        nc.tensor.matmul(
                            pq[:, a - tn * 512 : b - tn * 512],
                            u2t[:, wi : wi + 128],
                            q1[:, cic, a + dk - 3 : b + dk - 3],
                            start=(idx == 0),
                            stop=(idx == len(segs) - 1),
                        )
                    tq2 = sp.tile([128, 512], F32, tag="tq2")
                    nc.vector.tensor_scalar(
                        tq2[:], pq[:], c2t[:, coc : coc + 1], None,
                        mybir.AluOpType.add,
                    )
                    nc.vector.scalar_tensor_tensor(
                        q2[:, coc, tn * 512 : (tn + 1) * 512], tq2[:], LRELU, tq2[:],
                        mybir.AluOpType.mult, mybir.AluOpType.max,
                    )
            qf = sp.tile([80, TD], F32, tag="qf")
            for tn in range(2):
                pq3 = pp.tile([80, 512], F32, tag="psB")
                for cic in range(2):
                    nc.tensor.matmul(
                        pq3[:], u3t[:, cic * 80 : cic * 80 + 80],
                        q2[:, cic, tn * 512 : (tn + 1) * 512],
                        start=(cic == 0), stop=(cic == 1),
                    )
                nc.scalar.activation(
                    qf[:, tn * 512 : (tn + 1) * 512], pq3[:], AF.Identity,
                    bias=c3t[:, 0:1],
                )
            qsq = sp.tile([80, TD], F32, tag="qsq")
            nc.scalar.activation(qsq[:], qf[:], AF.Square)
            q2row = sp.tile([1, TD], F32, tag="q2row")
            for c in range(8):
                pqr = pp.tile([1, 512], F32, tag="psS")
                nc.tensor.matmul(
                    pqr[:, :128], o80t[:], qsq[:, c * 128 : (c + 1) * 128],
                    start=True, stop=True,
                )
                nc.scalar.activation(
                    q2row[:, c * 128 : (c + 1) * 128], pqr[:, :128], AF.Copy,
                    scale=-0.5,
                )

            # ---- distance + softmaxes per 128-row chunk of TD ----
            for c in range(8):
                pa = pp.tile([128, 512], F32, tag="psA")
                nc.tensor.matmul(
                    pa[:, :TE], qf[:, c * 128 : (c + 1) * 128], ke[:],
                    start=True, stop=False,
                )
                nc.tensor.matmul(
                    pa[:, :TE], o1t[:, :128], k2row[:], start=False, stop=False,
                )
                nc.tensor.matmul(
                    pa[:, :TE], q2row[:, c * 128 : (c + 1) * 128], o1t[:],
                    start=False, stop=True,
                )
                # pa = qk - q2/2 - k2/2 ; logits = 2*TEMP*pa
                e1 = sp.tile([128, TE], F32, tag="e1")
                se = sp.tile([128, 1], F32, tag="se")
                nc.scalar.activation(
                    e1[:], pa[:, :TE], AF.Exp, scale=2.0 * TEMP, accum_out=se[:]
                )
                ise = sp.tile([128, 1], F32, tag="ise")
                nc.vector.reciprocal(ise[:], se[:])
                nlz = sp.tile([128, 1], F32, tag="nlz")
                nc.scalar.activation(nlz[:], ise[:], AF.Ln)
                plog = sp.tile([128, TE], F32, tag="plog")
                nc.sync.dma_start(plog[:], priorT[s, c * 128 : (c + 1) * 128, :])
                tml = sp.tile([128, TE], F32, tag="tml")
                nc.scalar.activation(
                    tml[:], pa[:, :TE], AF.Identity, scale=2.0 * TEMP,
                    bias=nlz[:, 0:1],
                )
                alp = sp.tile([128, TE], F32, tag="alp")
                nc.vector.tensor_add(alp[:], tml[:], plog[:])
                nc.sync.dma_start(alp_out[s, c * 128 : (c + 1) * 128, :], alp[:])
                sexp = sp.tile([128, TE], F32, tag="sexp")
                ss = sp.tile([128, 1], F32, tag="ss")
                nc.scalar.activation(sexp[:], alp[:], AF.Exp, accum_out=ss[:])
                rs = sp.tile([128, 1], F32, tag="rs")
                nc.vector.reciprocal(rs[:], ss[:])
                asoft = sp.tile([128, TE], F32, tag="asoft")
                nc.scalar.activation(asoft[:], sexp[:], AF.Copy, scale=rs[:, 0:1])
                nc.sync.dma_start(
                    soft_out[s, c * 128 : (c + 1) * 128, :], asoft[:]
                )

    nc.compile()
    return nc


def _get_graph():
    global _GRAPH
    if _GRAPH is None:
        _GRAPH = _build_graph()
    return _GRAPH


def _mas_batch(la, in_lens, out_lens):
    """Vectorized-over-batch replica of the reference MAS (numpy).
    Takes log-probabilities directly (per-row constant offsets do not
    change the argmax path)."""
    Bn, Td, Te = la.shape
    cols = np.arange(Te)
    la = np.where(cols[None, None, :] < in_lens[:, None, None], la, -np.inf)
    la[:, 0, 1:] = -np.inf
    prev = np.zeros((Bn, Td, Te), np.int8)
    logp = la[:, 0, :].copy()
    ninf = np.full((Bn, 1), -np.inf, np.float32)
    gt0 = (cols[None, :] > 0)
    for i in range(1, Td):
        shifted = np.concatenate([ninf, logp[:, :-1]], axis=1)
        take = (shifted >= logp) & gt0
        prev[:, i] = take
        logp = la[:, i] + np.where(take, shifted, logp)
    opt = np.zeros((Bn, Td, Te), np.float32)
    curr = (in_lens.astype(np.int64) - 1).copy()
    bi = np.arange(Bn)
    for i in range(Td - 1, -1, -1):
        active = i < out_lens
        opt[bi, i, curr] = np.where(active, 1.0, opt[bi, i, curr])
        step = prev[bi, i, curr].astype(np.int64)
        curr = np.where(active, curr - step, curr)
    opt[:, 0, 0] = 1.0
    return opt


def kernel(**inputs):
    global LAST_EXEC_NS
    enc_in = np.ascontiguousarray(inputs["enc_in"], np.float32)
    dec_in = np.ascontiguousarray(inputs["dec_in"], np.float32)
    enc_len = np.asarray(inputs["enc_len"]).astype(np.int64)
    dec_len = np.asarray(inputs["dec_len"]).astype(np.int64)
    prior = np.ascontiguousarray(inputs["attn_prior"], np.float32)
    kw1 = np.asarray(inputs["kw1"], np.float32)
    kb1 = np.asarray(inputs["kb1"], np.float32)
    kw2 = np.asarray(inputs["kw2"], np.float32)
    kb2 = np.asarray(inputs["kb2"], np.float32)
    qw1 = np.asarray(inputs["qw1"], np.float32)
    qb1 = np.asarray(inputs["qb1"], np.float32)
    qw2 = np.asarray(inputs["qw2"], np.float32)
    qb2 = np.asarray(inputs["qb2"], np.float32)
    qw3 = np.asarray(inputs["qw3"], np.float32)
    qb3 = np.asarray(inputs["qb3"], np.float32)

    # host-side weight layout prep (replicated across cores)
    w1 = np.ascontiguousarray(
        kw1.transpose(2, 1, 0).reshape(3, 4, 128, 2, 128).transpose(2, 0, 1, 3, 4)
    ).reshape(128, -1)
    w2 = np.ascontiguousarray(
        kw2[:, :, 0].T.reshape(2, 128, 80).transpose(1, 0, 2)
    ).reshape(128, 160)
    u1 = np.ascontiguousarray(
        qw1.transpose(2, 1, 0).reshape(7, 80, 2, 128).transpose(1, 0, 2, 3)
    ).reshape(80, -1)
    u2 = np.ascontiguousarray(
        qw2.transpose(2, 1, 0).reshape(7, 2, 128, 2, 128).transpose(2, 0, 1, 3, 4)
    ).reshape(128, -1)
    u3 = np.ascontiguousarray(
        qw3[:, :, 0].T.reshape(2, 128, 80).transpose(1, 0, 2)
    ).reshape(128, 160)
    b1 = np.ascontiguousarray(kb1.reshape(2, 128).T)
    b2 = kb2.reshape(80, 1)
    c1 = np.ascontiguousarray(qb1.reshape(2, 128).T)
    c2 = np.ascontiguousarray(qb2.reshape(2, 128).T)
    c3 = qb3.reshape(80, 1)
    o80 = np.ones((80, 1), np.float32)
    o1r = np.ones((1, TE), np.float32)

    shared = dict(w1=w1, b1=b1, w2=w2, b2=b2, u1=u1, c1=c1, u2=u2, c2=c2,
                  u3=u3, c3=c3, o80=o80, o1r=o1r)
    in_maps = []
    for i in range(NCORES):
        sl = slice(i * BS, (i + 1) * BS)
        in_maps.append(dict(
            enc=np.ascontiguousarray(enc_in[sl].reshape(BS, 4, 128, TE)),
            dec=np.ascontiguousarray(dec_in[sl]),
            priorT=np.ascontiguousarray(np.log(prior[sl] + 1e-8).transpose(0, 2, 1)),
            **shared,
        ))

    nc = _get_graph()
    trace = bool(os.environ.get("KERNEL_TRACE"))
    res = run_bass_kernel_spmd(nc, in_maps, core_ids=list(range(NCORES)),
                               trace=trace)
    LAST_EXEC_NS = res.exec_time_ns
    outs = res.results
    alp = np.concatenate([np.asarray(r["alp"]) for r in outs], axis=0)
    soft = np.concatenate([np.asarray(r["soft"]) for r in outs], axis=0)
    alp = alp.reshape(B, 1, TD, TE).astype(np.float32)
    soft = soft.reshape(B, 1, TD, TE).astype(np.float32)

    alp32 = alp[:, 0]
    m = alp32.max(-1, keepdims=True)
    with np.errstate(over="ignore"):
        lse = m + np.log(np.exp(alp32 - m).sum(-1, keepdims=True))
    la = (alp32 - lse).astype(np.float32)
    hard = _mas_batch(la, enc_len, dec_len)[:, None]
    dur = hard.sum(axis=2)[:, 0, :]
    return alp, soft, hard, dur
